# revision 24
# baseline (speedup 1.0000x reference)
"""Multi-head causal attention (B=2, S=2048, D=1024, H=16) on 8 TRN2 cores.

Sharding (Megatron-style): core c handles batch b = c//4, head-group
g = c%4 (4 heads, d' slice of 256). Each core computes its partial
out = ctx_g @ Wo[gslice] (no bias) in fp16; host sums the 4 partials
per batch in fp32 and adds the bias.

Device kernel dataflow (fp16 matmul operands, fp32 PSUM accumulation):
  qT/kT [d', S] and v via PE projections ->
  scores sT[k, q] per (head-pair, k-tile, q-block) (row-packed K=64
  matmul pairs) -> exp on ACT (psum->sbuf, fused 1/sqrt(hd) scale) ->
  causal triangle masking via a DVE multiply with a precomputed 0/1
  triangle (fast, keeps GPSIMD off the critical path) ->
  ctxT + softmax denominators accumulated on PE (ones columns
  interleaved in the v operand) -> normalize with a single DVE divide
  (shifted-partition in1, psum/psum) -> out projection on PE.

Schedule: the scores->exp->ctx chain is software-pipelined (ctx for
k-tile i is emitted after scores for k-tile i+1, so the ACT exp of
tile i overlaps the PE scores of tile i+1), and projection / out-proj
matmul chains are drip-fed between attention steps from a filler
queue so the PE never starves while ACT chews exp. x is DMA'd
S-major (per q-block) so the first projection starts after ~1MB.
"""

import numpy as np

import concourse.bass as bass
import concourse.mybir as mybir
import concourse.tile as tile
from concourse import bacc, bass_utils
from concourse.hw_specs import get_activation_tables

F32 = mybir.dt.float32
F16 = mybir.dt.float16
EXP = mybir.ActivationFunctionType.Exp
LN = mybir.ActivationFunctionType.Ln

B, S, D, H, HD = 2, 2048, 1024, 16, 64
NHL = 4          # local heads per core
DC = NHL * HD    # 256 local d'
NDT = D // 128   # 8 contraction tiles for projections
KT = 128         # k tile
NKT = S // KT    # 16
QB = 512         # q block
NQB = S // QB    # 4
SCALE = 1.0 / np.sqrt(HD)

# va free-layout per k-tile m: [v0 | J | v1 | v2 | J | v3], J = ones(64)
# (matmul weight APs allow only one free dim, so the ones blocks are
# interleaved to make every head a contiguous 128-col slice).
# Head h reads 128 cols at VA_OFF[h]; even heads are [v|J] (ctx psum rows
# 0:64, denom 64:128), odd heads [J|v] (denom 0:64, ctx 64:128).
VA_W = 384
VA_OFF = [0, 64, 192, 256]


def _pin_act_table(arch):
    """Steer Bacc's activation-table chooser to the one set containing both
    exp and ln, so interleaved Exp/Ln calls don't thrash ACT_TABLE_LOADs
    (measured 1.28us per reload)."""
    tabs = get_activation_tables(arch)
    keep = "natural_log_exp_and_others"
    if keep not in tabs:
        return
    for name, funcs in tabs.items():
        if name != keep:
            funcs.discard(EXP)
            funcs.discard(LN)


class _Filler:
    """FIFO of (key, generator) PE filler tasks. pop(n) advances n units;
    flush(key) force-finishes every generator registered under key."""

    def __init__(self):
        self.q = []

    def add(self, key, gen):
        self.q.append([key, gen])

    def pop(self, n):
        done = 0
        while done < n and self.q:
            try:
                next(self.q[0][1])
                done += 1
            except StopIteration:
                self.q.pop(0)

    def flush(self, key):
        for item in [it for it in self.q if it[0] == key]:
            for _ in item[1]:
                pass
            self.q.remove(item)

    def drain(self):
        for _, g in self.q:
            for _ in g:
                pass
        self.q = []


def build_nc():
    nc = bacc.Bacc("TRN2", target_bir_lowering=False, debug=False)
    xT = nc.dram_tensor("xT", [128, NQB, NDT, QB], F16, kind="ExternalInput")
    wq = nc.dram_tensor("wq", [128, NDT, DC], F16, kind="ExternalInput")
    wk = nc.dram_tensor("wk", [128, NDT, DC], F16, kind="ExternalInput")
    wv = nc.dram_tensor("wv", [128, NDT, DC], F16, kind="ExternalInput")
    wo = nc.dram_tensor("wo", [128, 2, D], F16, kind="ExternalInput")
    out = nc.dram_tensor("out_p", [S, D], F16, kind="ExternalOutput")

    with tile.TileContext(nc) as tc:
        with (
            tc.tile_pool(name="xp", bufs=1) as xp,
            tc.tile_pool(name="wp", bufs=1) as wp,
            tc.tile_pool(name="qk", bufs=1) as qkp,
            tc.tile_pool(name="vap", bufs=1) as vap,
            tc.tile_pool(name="cnp", bufs=1) as cnp,
            tc.tile_pool(name="et", bufs=8) as etp,
            tc.tile_pool(name="rcp", bufs=2) as rcp,
            tc.tile_pool(name="ob", bufs=4) as obp,
            tc.tile_pool(name="pp", bufs=2, space="PSUM") as pp,
            tc.tile_pool(name="sp", bufs=2, space="PSUM") as sp,
            tc.tile_pool(name="cp", bufs=2, space="PSUM") as cp,
        ):
            # ---- loads: weights on the scalar HWDGE queue, x S-major
            # (per q-block) on the sync queue so block-0 projections can
            # start after ~1MB instead of the full 4MB.
            wq_sb = wp.tile([128, NDT, DC], F16, tag="wq")
            wk_sb = wp.tile([128, NDT, DC], F16, tag="wk")
            wv_sb = wp.tile([128, NDT, DC], F16, tag="wv")
            wo_sb = wp.tile([128, 2, D], F16, tag="wo")
            x_sb = xp.tile([128, NQB, NDT, QB], F16)
            nc.scalar.dma_start(wq_sb[:], wq.ap())
            nc.scalar.dma_start(wk_sb[:], wk.ap())
            for j in range(NQB):
                nc.sync.dma_start(x_sb[:, j], xT.ap()[:, j])
            nc.scalar.dma_start(wv_sb[:], wv.ap())
            nc.scalar.dma_start(wo_sb[:], wo.ap())

            def xt(t, j):
                # x block j, contraction tile t: [128, QB]
                return x_sb[:, j, t, :]

            # ---- constants; also preload the ACT table early ----
            # Warmup operand comes from a DVE memset so the warmup matmuls
            # don't wait for the GPSIMD iram load + tri construction.
            wu16 = wp.tile([128, 128], F16, tag="wu16")
            nc.vector.memset(wu16[:], 1.0)
            tri = wp.tile([128, 128], F16, tag="tri")
            nc.gpsimd.memset(tri[:], 1.0)
            # tri[k, q] = 1 if q >= k else 0
            nc.gpsimd.affine_select(
                out=tri[:], in_=tri[:], compare_op=mybir.AluOpType.is_ge,
                fill=0.0, base=0, pattern=[[1, 128]], channel_multiplier=-1)
            # Keep the PE busy while input DMAs stream so the HAM clock
            # gate reaches 8/8 before the projection chains start.
            wps = sp.tile([128, 128], F32, tag="sp", name="wps")
            for _ in range(48):
                nc.tensor.matmul(wps[:], wu16[:], wu16[:], start=True, stop=True)
            va = vap.tile([128, NKT, VA_W], F16)
            nc.gpsimd.memset(va[:], 1.0)
            warmup = wp.tile([1, 8], F32, tag="wu")
            nc.vector.memset(warmup[:], 1.0)
            nc.scalar.activation(warmup[:], warmup[:], EXP, scale=0.001)

            # ---- persistent SBUF tensors ----
            qT = [qkp.tile([128, S], F16, tag=f"q{hp}", name=f"q{hp}")
                  for hp in range(2)]
            kTt = [qkp.tile([128, S], F16, tag=f"k{hp}", name=f"k{hp}")
                   for hp in range(2)]
            ctxn = [cnp.tile([128, S], F16, tag=f"c{t}", name=f"c{t}")
                    for t in range(2)]

            # ---- filler generators (each yield = one PE chain unit) ----
            def gen_proj_qk(hp, j):
                for dst, w_sb in ((qT[hp], wq_sb), (kTt[hp], wk_sb)):
                    ps = pp.tile([128, QB], F32, tag="pp")
                    for t in range(NDT):
                        nc.tensor.matmul(
                            ps[:], w_sb[:, t, 128 * hp:128 * (hp + 1)],
                            xt(t, j), start=(t == 0), stop=(t == NDT - 1))
                    nc.vector.tensor_copy(dst[:, QB * j:QB * (j + 1)], ps[:])
                    yield

            def gen_v(j):
                for m in range(4 * j, 4 * (j + 1)):
                    ps = pp.tile([128, DC], F32, tag="pp")
                    mo = 128 * (m % 4)
                    for t in range(NDT):
                        nc.tensor.matmul(
                            ps[:], xt(t, j)[:, mo:mo + 128],
                            wv_sb[:, t, :], start=(t == 0), stop=(t == NDT - 1))
                    # [v0|J|v1|v2|J|v3]: 3 contiguous copies
                    nc.vector.tensor_copy(va[:, m, 0:64], ps[:, 0:64])
                    nc.vector.tensor_copy(va[:, m, 128:256], ps[:, 64:192])
                    nc.vector.tensor_copy(va[:, m, 320:384], ps[:, 192:256])
                    yield

            def gen_out(mlo, mhi, pool=None, ptag="pp"):
                pool = pool or pp
                for m in range(mlo, mhi):
                    ot = obp.tile([128, D], F16, tag="ob")
                    for o in range(2):
                        ps = pool.tile([128, QB], F32, tag=ptag)
                        for t in range(2):
                            nc.tensor.matmul(
                                ps[:], ctxn[t][:, 128 * m:128 * (m + 1)],
                                wo_sb[:, t, QB * o:QB * (o + 1)],
                                start=(t == 0), stop=(t == 1))
                        nc.vector.tensor_copy(ot[:, QB * o:QB * (o + 1)], ps[:])
                        yield
                    nc.sync.dma_start(out.ap()[128 * m:128 * (m + 1), :], ot[:])

            # ---- attention block, ctx lagging scores by one k-tile ----
            def attention_block(hp, j, filler):
                nkt_j = 4 * j + 4
                cpe = cp.tile([128, QB], F32, tag="cp")   # head 2hp
                cpo = cp.tile([128, QB], F32, tag="cp")   # head 2hp+1

                def emit_ctx(i, et, c0):
                    for lh, cpt in ((0, cpe), (1, cpo)):
                        h = 2 * hp + lh
                        nc.tensor.matmul(
                            cpt[:, c0:QB],
                            va[:, i, VA_OFF[h]:VA_OFF[h] + 128],
                            et[:, lh, c0:QB],
                            start=(i == 0), stop=(i == nkt_j - 1))

                pend = []
                for i in range(nkt_j):
                    c0 = max(0, 128 * (i - 4 * j))
                    spt = sp.tile([128, 2, QB], F32, tag="sp")
                    for lh in range(2):
                        nc.tensor.matmul(
                            spt[:, lh, c0:QB],
                            kTt[hp][64 * lh:64 * (lh + 1), 128 * i:128 * (i + 1)],
                            qT[hp][64 * lh:64 * (lh + 1), QB * j + c0:QB * (j + 1)],
                            start=True, stop=True)
                    et = etp.tile([128, 2, QB], F16, tag="et")
                    # one (possibly strided) exp for both heads
                    nc.scalar.activation(et[:, :, c0:QB], spt[:, :, c0:QB],
                                         EXP, scale=float(SCALE))
                    if i >= 4 * j:  # diagonal: triangle mask in place
                        for lh in range(2):
                            sl = et[:, lh, c0:c0 + 128]
                            nc.gpsimd.affine_select(
                                out=sl, in_=sl,
                                compare_op=mybir.AluOpType.is_ge, fill=0.0,
                                base=0, pattern=[[1, 128]],
                                channel_multiplier=-1)
                    pend.append((i, et, c0))
                    if len(pend) > 1:
                        emit_ctx(*pend.pop(0))
                    filler.pop(1)
                while pend:
                    emit_ctx(*pend.pop(0))
                # normalize; gather both heads' denominators into one sbuf
                # tile (rows 0:64 <- cpe rows 64:128, rows 64:128 <- cpo
                # rows 0:64), one ln + one exp(-x) on ACT for 1/denom, then
                # two aligned-partition DVE muls. (DVE InstReciprocal
                # measured 3.35us/op; the custom-DVE reciprocal_approx_fast
                # returns garbage on HW in this kernel - both rejected.)
                dsb = rcp.tile([128, QB], F32, tag="t1")
                rc = rcp.tile([128, QB], F32, tag="rc")
                nc.vector.tensor_copy(dsb[0:64, :], cpe[64:128, :])
                nc.vector.tensor_copy(dsb[64:128, :], cpo[0:64, :])
                nc.scalar.activation(rc[:], dsb[:], LN)
                nc.scalar.activation(dsb[:], rc[:], EXP, scale=-1.0)
                for lh, cpt in ((0, cpe), (1, cpo)):
                    cr = slice(64 * lh, 64 * lh + 64)        # ctx rows
                    nc.vector.tensor_mul(
                        ctxn[hp][cr, QB * j:QB * (j + 1)], cpt[cr, :],
                        dsb[cr, :])

            # ---- schedule ----
            filler = _Filler()
            # lead-in: pair-0 block-0 projections + first v tiles inline
            for _ in gen_proj_qk(0, 0):
                pass
            for _ in gen_v(0):
                pass
            # out-proj chunks for block jb are queued two j-blocks later
            # (clamped to the last block) so the long final attention
            # blocks - which have the most ACT exp work - get the most PE
            # filler. Block NQB-2's chunks are held back entirely: they are
            # emitted right after the last attention block, where they hide
            # the final normalize chain latency (and keep the PE p-state
            # up for the dependent final chunks).
            def outs_at(j):
                return [jb for jb in range(NQB - 1)
                        if min(jb + 2, NQB - 1) == j and jb != NQB - 2]

            for j in range(NQB):
                filler.add(("qk", 1, j), gen_proj_qk(1, j))
                for jb in outs_at(j):
                    filler.add(("out", jb, 0), gen_out(4 * jb, 4 * jb + 2))
                filler.flush(("qk", 0, j))
                filler.flush(("v", j))
                attention_block(0, j, filler)
                if j + 1 < NQB:
                    filler.add(("qk", 0, j + 1), gen_proj_qk(0, j + 1))
                    filler.add(("v", j + 1), gen_v(j + 1))
                for jb in outs_at(j):
                    filler.add(("out", jb, 1), gen_out(4 * jb + 2, 4 * jb + 4))
                filler.flush(("qk", 1, j))
                attention_block(1, j, filler)
            filler.drain()
            if NQB >= 2:  # held-back block NQB-2: ready filler, hides norm
                for _ in gen_out(4 * NQB - 8, 4 * NQB - 4):
                    pass
            # final block: alternate psum pools per chunk (sp is idle by
            # now) so the chunk pipeline spreads across 4 banks
            for m in range(4 * NQB - 4, 4 * NQB):
                pool, ptag = ((sp, "sp") if m % 2 == 0 else (pp, "pp"))
                for _ in gen_out(m, m + 1, pool=pool, ptag=ptag):
                    pass
    _pin_act_table(nc.m.arch)
    nc.compile()
    return nc


_NC = None


def _get_nc():
    global _NC
    if _NC is None:
        _NC = build_nc()
    return _NC


def make_in_maps(x, Wq, Wk, Wv, Wo):
    x = np.asarray(x, np.float32)
    Wq, Wk, Wv, Wo = (np.asarray(w, np.float32) for w in (Wq, Wk, Wv, Wo))
    in_maps = []
    for c in range(8):
        b, g = c // 4, c % 4
        sl = slice(DC * g, DC * (g + 1))
        in_maps.append({
            "xT": np.ascontiguousarray(
                x[b].T.astype(np.float16).reshape(NDT, 128, NQB, QB)
                .transpose(1, 2, 0, 3)),
            "wq": np.ascontiguousarray(
                Wq[:, sl].astype(np.float16).reshape(NDT, 128, DC)
                .transpose(1, 0, 2)),
            "wk": np.ascontiguousarray(
                Wk[:, sl].astype(np.float16).reshape(NDT, 128, DC)
                .transpose(1, 0, 2)),
            "wv": np.ascontiguousarray(
                Wv[:, sl].astype(np.float16).reshape(NDT, 128, DC)
                .transpose(1, 0, 2)),
            "wo": np.ascontiguousarray(
                Wo[sl, :].astype(np.float16).reshape(2, 128, D)
                .transpose(1, 0, 2)),
        })
    return in_maps


def kernel(x, Wq, Wk, Wv, Wo, bo, _trace=False, _trace_cores=None):
    nc = _get_nc()
    in_maps = make_in_maps(x, Wq, Wk, Wv, Wo)
    res = bass_utils.run_bass_kernel_spmd(
        nc, in_maps, core_ids=list(range(8)), trace=_trace,
        trace_cores=_trace_cores)
    bo = np.asarray(bo, np.float32)
    out = np.empty((B, S, D), np.float32)
    for b in range(B):
        acc = res.results[4 * b]["out_p"].astype(np.float32)
        for g in range(1, 4):
            acc += res.results[4 * b + g]["out_p"].astype(np.float32)
        out[b] = acc + bo
    kernel.last_results = res
    return out


# revision 28
# speedup vs baseline: 1.0210x; 1.0210x over previous
"""Multi-head causal attention (B=2, S=2048, D=1024, H=16) on 8 TRN2 cores.

Sharding (Megatron-style): core c handles batch b = c//4, head-group
g = c%4 (4 heads, d' slice of 256). Each core computes its partial
out = ctx_g @ Wo[gslice] (no bias) in fp16; host sums the 4 partials
per batch in fp32 and adds the bias.

Device kernel dataflow (fp16 matmul operands, fp32 PSUM accumulation):
  qT/kT [d', S] and v via PE projections ->
  scores sT[k, q] per (head-pair, k-tile, q-block) (row-packed K=64
  matmul pairs) -> exp on ACT (psum->sbuf, fused 1/sqrt(hd) scale) ->
  causal triangle masking via a DVE multiply with a precomputed 0/1
  triangle (fast, keeps GPSIMD off the critical path) ->
  ctxT + softmax denominators accumulated on PE (ones columns
  interleaved in the v operand) -> normalize with a single DVE divide
  (shifted-partition in1, psum/psum) -> out projection on PE.

Schedule: the scores->exp->ctx chain is software-pipelined (ctx for
k-tile i is emitted after scores for k-tile i+1, so the ACT exp of
tile i overlaps the PE scores of tile i+1), and projection / out-proj
matmul chains are drip-fed between attention steps from a filler
queue so the PE never starves while ACT chews exp. x is DMA'd
S-major (per q-block) so the first projection starts after ~1MB.
"""

import numpy as np

import concourse.bass as bass
import concourse.mybir as mybir
import concourse.tile as tile
from concourse import bacc, bass_utils
from concourse.hw_specs import get_activation_tables

F32 = mybir.dt.float32
F16 = mybir.dt.float16
EXP = mybir.ActivationFunctionType.Exp
LN = mybir.ActivationFunctionType.Ln

B, S, D, H, HD = 2, 2048, 1024, 16, 64
NHL = 4          # local heads per core
DC = NHL * HD    # 256 local d'
NDT = D // 128   # 8 contraction tiles for projections
KT = 128         # k tile
NKT = S // KT    # 16
QB = 512         # q block
NQB = S // QB    # 4
SCALE = 1.0 / np.sqrt(HD)

# va free-layout per k-tile m: [v0 | J | v1 | v2 | J | v3], J = ones(64)
# (matmul weight APs allow only one free dim, so the ones blocks are
# interleaved to make every head a contiguous 128-col slice).
# Head h reads 128 cols at VA_OFF[h]; even heads are [v|J] (ctx psum rows
# 0:64, denom 64:128), odd heads [J|v] (denom 0:64, ctx 64:128).
VA_W = 384
VA_OFF = [0, 64, 192, 256]


def _pin_act_table(arch):
    """Steer Bacc's activation-table chooser to the one set containing both
    exp and ln, so interleaved Exp/Ln calls don't thrash ACT_TABLE_LOADs
    (measured 1.28us per reload)."""
    tabs = get_activation_tables(arch)
    keep = "natural_log_exp_and_others"
    if keep not in tabs:
        return
    for name, funcs in tabs.items():
        if name != keep:
            funcs.discard(EXP)
            funcs.discard(LN)


class _Filler:
    """FIFO of (key, generator) PE filler tasks. pop(n) advances n units;
    flush(key) force-finishes every generator registered under key."""

    def __init__(self):
        self.q = []

    def add(self, key, gen):
        self.q.append([key, gen])

    def pop(self, n):
        done = 0
        while done < n and self.q:
            try:
                next(self.q[0][1])
                done += 1
            except StopIteration:
                self.q.pop(0)

    def flush(self, key):
        for item in [it for it in self.q if it[0] == key]:
            for _ in item[1]:
                pass
            self.q.remove(item)

    def drain(self):
        for _, g in self.q:
            for _ in g:
                pass
        self.q = []


def build_nc():
    nc = bacc.Bacc("TRN2", target_bir_lowering=False, debug=False)
    xT = nc.dram_tensor("xT", [128, NQB, NDT, QB], F16, kind="ExternalInput")
    wq = nc.dram_tensor("wq", [128, NDT, DC], F16, kind="ExternalInput")
    wk = nc.dram_tensor("wk", [128, NDT, DC], F16, kind="ExternalInput")
    wv = nc.dram_tensor("wv", [128, NDT, DC], F16, kind="ExternalInput")
    wo = nc.dram_tensor("wo", [128, 2, D], F16, kind="ExternalInput")
    out = nc.dram_tensor("out_p", [S, D], F16, kind="ExternalOutput")

    with tile.TileContext(nc) as tc:
        with (
            tc.tile_pool(name="xp", bufs=1) as xp,
            tc.tile_pool(name="wp", bufs=1) as wp,
            tc.tile_pool(name="qk", bufs=1) as qkp,
            tc.tile_pool(name="vap", bufs=1) as vap,
            tc.tile_pool(name="cnp", bufs=1) as cnp,
            tc.tile_pool(name="et", bufs=8) as etp,
            tc.tile_pool(name="rcp", bufs=2) as rcp,
            tc.tile_pool(name="ob", bufs=4) as obp,
            tc.tile_pool(name="pp", bufs=2, space="PSUM") as pp,
            tc.tile_pool(name="sp", bufs=2, space="PSUM") as sp,
            tc.tile_pool(name="cp", bufs=2, space="PSUM") as cp,
        ):
            # ---- loads: weights on the scalar HWDGE queue, x S-major
            # (per q-block) on the sync queue so block-0 projections can
            # start after ~1MB instead of the full 4MB.
            wq_sb = wp.tile([128, NDT, DC], F16, tag="wq")
            wk_sb = wp.tile([128, NDT, DC], F16, tag="wk")
            wv_sb = wp.tile([128, NDT, DC], F16, tag="wv")
            wo_sb = wp.tile([128, 2, D], F16, tag="wo")
            # Spread the startup-critical loads across four engine DMA
            # queues: each queue has ~10us bring-up latency and only
            # ~55-130GB/s, so serialising x+weights on two queues left the
            # PE starved until ~26us. Block-0 x is split across two queues.
            x_sb = xp.tile([128, NQB, NDT, QB], F16)
            nc.scalar.dma_start(wq_sb[:], wq.ap())
            nc.scalar.dma_start(wk_sb[:], wk.ap())
            nc.sync.dma_start(x_sb[:, 0, 0:4], xT.ap()[:, 0, 0:4])
            nc.gpsimd.dma_start(x_sb[:, 0, 4:8], xT.ap()[:, 0, 4:8])
            nc.gpsimd.dma_start(wv_sb[:], wv.ap())
            for j in range(1, NQB):
                eng = nc.sync if j % 2 else nc.gpsimd
                eng.dma_start(x_sb[:, j], xT.ap()[:, j])
            nc.scalar.dma_start(wo_sb[:], wo.ap())

            def xt(t, j):
                # x block j, contraction tile t: [128, QB]
                return x_sb[:, j, t, :]

            # ---- constants; also preload the ACT table early ----
            # Warmup operand comes from a DVE memset so the warmup matmuls
            # don't wait for the GPSIMD iram load + tri construction.
            wu16 = wp.tile([128, 128], F16, tag="wu16")
            nc.vector.memset(wu16[:], 1.0)
            tri = wp.tile([128, 128], F16, tag="tri")
            nc.gpsimd.memset(tri[:], 1.0)
            # tri[k, q] = 1 if q >= k else 0
            nc.gpsimd.affine_select(
                out=tri[:], in_=tri[:], compare_op=mybir.AluOpType.is_ge,
                fill=0.0, base=0, pattern=[[1, 128]], channel_multiplier=-1)
            # Keep the PE busy while input DMAs stream so the HAM clock
            # gate reaches 8/8 before the projection chains start.
            wps = sp.tile([128, 128], F32, tag="sp", name="wps")
            for _ in range(64):
                nc.tensor.matmul(wps[:], wu16[:], wu16[:], start=True, stop=True)
            va = vap.tile([128, NKT, VA_W], F16)
            nc.gpsimd.memset(va[:], 1.0)
            warmup = wp.tile([1, 8], F32, tag="wu")
            nc.vector.memset(warmup[:], 1.0)
            nc.scalar.activation(warmup[:], warmup[:], EXP, scale=0.001)

            # ---- persistent SBUF tensors ----
            qT = [qkp.tile([128, S], F16, tag=f"q{hp}", name=f"q{hp}")
                  for hp in range(2)]
            kTt = [qkp.tile([128, S], F16, tag=f"k{hp}", name=f"k{hp}")
                   for hp in range(2)]
            ctxn = [cnp.tile([128, S], F16, tag=f"c{t}", name=f"c{t}")
                    for t in range(2)]

            # ---- filler generators (each yield = one PE chain unit) ----
            def gen_proj_qk(hp, j):
                for dst, w_sb in ((qT[hp], wq_sb), (kTt[hp], wk_sb)):
                    ps = pp.tile([128, QB], F32, tag="pp")
                    for t in range(NDT):
                        nc.tensor.matmul(
                            ps[:], w_sb[:, t, 128 * hp:128 * (hp + 1)],
                            xt(t, j), start=(t == 0), stop=(t == NDT - 1))
                    nc.vector.tensor_copy(dst[:, QB * j:QB * (j + 1)], ps[:])
                    yield

            def gen_v(j):
                for m in range(4 * j, 4 * (j + 1)):
                    ps = pp.tile([128, DC], F32, tag="pp")
                    mo = 128 * (m % 4)
                    for t in range(NDT):
                        nc.tensor.matmul(
                            ps[:], xt(t, j)[:, mo:mo + 128],
                            wv_sb[:, t, :], start=(t == 0), stop=(t == NDT - 1))
                    # [v0|J|v1|v2|J|v3]: 3 contiguous copies
                    nc.vector.tensor_copy(va[:, m, 0:64], ps[:, 0:64])
                    nc.vector.tensor_copy(va[:, m, 128:256], ps[:, 64:192])
                    nc.vector.tensor_copy(va[:, m, 320:384], ps[:, 192:256])
                    yield

            def gen_out(mlo, mhi, pool=None, ptag="pp"):
                pool = pool or pp
                for m in range(mlo, mhi):
                    ot = obp.tile([128, D], F16, tag="ob")
                    for o in range(2):
                        ps = pool.tile([128, QB], F32, tag=ptag)
                        for t in range(2):
                            nc.tensor.matmul(
                                ps[:], ctxn[t][:, 128 * m:128 * (m + 1)],
                                wo_sb[:, t, QB * o:QB * (o + 1)],
                                start=(t == 0), stop=(t == 1))
                        nc.vector.tensor_copy(ot[:, QB * o:QB * (o + 1)], ps[:])
                        yield
                    oeng = nc.sync if m % 2 else nc.gpsimd
                    oeng.dma_start(out.ap()[128 * m:128 * (m + 1), :], ot[:])

            # ---- attention block, ctx lagging scores by one k-tile ----
            def attention_block(hp, j, filler):
                nkt_j = 4 * j + 4
                cpe = cp.tile([128, QB], F32, tag="cp")   # head 2hp
                cpo = cp.tile([128, QB], F32, tag="cp")   # head 2hp+1

                def emit_ctx(i, et, c0):
                    for lh, cpt in ((0, cpe), (1, cpo)):
                        h = 2 * hp + lh
                        nc.tensor.matmul(
                            cpt[:, c0:QB],
                            va[:, i, VA_OFF[h]:VA_OFF[h] + 128],
                            et[:, lh, c0:QB],
                            start=(i == 0), stop=(i == nkt_j - 1))

                pend = []
                for i in range(nkt_j):
                    c0 = max(0, 128 * (i - 4 * j))
                    spt = sp.tile([128, 2, QB], F32, tag="sp")
                    for lh in range(2):
                        nc.tensor.matmul(
                            spt[:, lh, c0:QB],
                            kTt[hp][64 * lh:64 * (lh + 1), 128 * i:128 * (i + 1)],
                            qT[hp][64 * lh:64 * (lh + 1), QB * j + c0:QB * (j + 1)],
                            start=True, stop=True)
                    et = etp.tile([128, 2, QB], F16, tag="et")
                    # one (possibly strided) exp for both heads
                    nc.scalar.activation(et[:, :, c0:QB], spt[:, :, c0:QB],
                                         EXP, scale=float(SCALE))
                    if i >= 4 * j:  # diagonal: triangle mask in place
                        for lh in range(2):
                            sl = et[:, lh, c0:c0 + 128]
                            nc.gpsimd.affine_select(
                                out=sl, in_=sl,
                                compare_op=mybir.AluOpType.is_ge, fill=0.0,
                                base=0, pattern=[[1, 128]],
                                channel_multiplier=-1)
                    pend.append((i, et, c0))
                    if len(pend) > 1:
                        emit_ctx(*pend.pop(0))
                    filler.pop(1)
                while pend:
                    emit_ctx(*pend.pop(0))
                # normalize; gather both heads' denominators into one sbuf
                # tile (rows 0:64 <- cpe rows 64:128, rows 64:128 <- cpo
                # rows 0:64), one ln + one exp(-x) on ACT for 1/denom, then
                # two aligned-partition DVE muls. (DVE InstReciprocal
                # measured 3.35us/op; the custom-DVE reciprocal_approx_fast
                # returns garbage on HW in this kernel - both rejected.)
                dsb = rcp.tile([128, QB], F32, tag="t1")
                rc = rcp.tile([128, QB], F32, tag="rc")
                nc.vector.tensor_copy(dsb[0:64, :], cpe[64:128, :])
                nc.vector.tensor_copy(dsb[64:128, :], cpo[0:64, :])
                nc.scalar.activation(rc[:], dsb[:], LN)
                nc.scalar.activation(dsb[:], rc[:], EXP, scale=-1.0)
                for lh, cpt in ((0, cpe), (1, cpo)):
                    cr = slice(64 * lh, 64 * lh + 64)        # ctx rows
                    nc.vector.tensor_mul(
                        ctxn[hp][cr, QB * j:QB * (j + 1)], cpt[cr, :],
                        dsb[cr, :])

            # ---- schedule ----
            filler = _Filler()
            # lead-in: pair-0 block-0 projections + first v tiles inline
            for _ in gen_proj_qk(0, 0):
                pass
            for _ in gen_v(0):
                pass
            # out-proj chunks for block jb are queued two j-blocks later
            # (clamped to the last block) so the long final attention
            # blocks - which have the most ACT exp work - get the most PE
            # filler. Block NQB-2's chunks are held back entirely: they are
            # emitted right after the last attention block, where they hide
            # the final normalize chain latency (and keep the PE p-state
            # up for the dependent final chunks).
            def outs_at(j):
                return [jb for jb in range(NQB - 1)
                        if min(jb + 2, NQB - 1) == j and jb != NQB - 2]

            for j in range(NQB):
                filler.add(("qk", 1, j), gen_proj_qk(1, j))
                for jb in outs_at(j):
                    filler.add(("out", jb, 0), gen_out(4 * jb, 4 * jb + 2))
                filler.flush(("qk", 0, j))
                filler.flush(("v", j))
                attention_block(0, j, filler)
                if j + 1 < NQB:
                    filler.add(("qk", 0, j + 1), gen_proj_qk(0, j + 1))
                    filler.add(("v", j + 1), gen_v(j + 1))
                for jb in outs_at(j):
                    filler.add(("out", jb, 1), gen_out(4 * jb + 2, 4 * jb + 4))
                filler.flush(("qk", 1, j))
                attention_block(1, j, filler)
            filler.drain()
            if NQB >= 2:  # held-back block NQB-2: ready filler, hides norm
                for _ in gen_out(4 * NQB - 8, 4 * NQB - 4):
                    pass
            # final block: alternate psum pools per chunk (sp is idle by
            # now) so the chunk pipeline spreads across 4 banks
            for m in range(4 * NQB - 4, 4 * NQB):
                pool, ptag = ((sp, "sp") if m % 2 == 0 else (pp, "pp"))
                for _ in gen_out(m, m + 1, pool=pool, ptag=ptag):
                    pass
    _pin_act_table(nc.m.arch)
    nc.compile()
    return nc


_NC = None


def _get_nc():
    global _NC
    if _NC is None:
        _NC = build_nc()
    return _NC


def make_in_maps(x, Wq, Wk, Wv, Wo):
    x = np.asarray(x, np.float32)
    Wq, Wk, Wv, Wo = (np.asarray(w, np.float32) for w in (Wq, Wk, Wv, Wo))
    in_maps = []
    for c in range(8):
        b, g = c // 4, c % 4
        sl = slice(DC * g, DC * (g + 1))
        in_maps.append({
            "xT": np.ascontiguousarray(
                x[b].T.astype(np.float16).reshape(NDT, 128, NQB, QB)
                .transpose(1, 2, 0, 3)),
            "wq": np.ascontiguousarray(
                Wq[:, sl].astype(np.float16).reshape(NDT, 128, DC)
                .transpose(1, 0, 2)),
            "wk": np.ascontiguousarray(
                Wk[:, sl].astype(np.float16).reshape(NDT, 128, DC)
                .transpose(1, 0, 2)),
            "wv": np.ascontiguousarray(
                Wv[:, sl].astype(np.float16).reshape(NDT, 128, DC)
                .transpose(1, 0, 2)),
            "wo": np.ascontiguousarray(
                Wo[sl, :].astype(np.float16).reshape(2, 128, D)
                .transpose(1, 0, 2)),
        })
    return in_maps


def kernel(x, Wq, Wk, Wv, Wo, bo, _trace=False, _trace_cores=None):
    nc = _get_nc()
    in_maps = make_in_maps(x, Wq, Wk, Wv, Wo)
    res = bass_utils.run_bass_kernel_spmd(
        nc, in_maps, core_ids=list(range(8)), trace=_trace,
        trace_cores=_trace_cores)
    bo = np.asarray(bo, np.float32)
    out = np.empty((B, S, D), np.float32)
    for b in range(B):
        acc = res.results[4 * b]["out_p"].astype(np.float32)
        for g in range(1, 4):
            acc += res.results[4 * b + g]["out_p"].astype(np.float32)
        out[b] = acc + bo
    kernel.last_results = res
    return out
